# revision 19
# baseline (speedup 1.0000x reference)
"""Trainium2 Bass kernel for the ragged 2-layer GRU problem (nn_DeepIOFeat11).

Contract: kernel(**inputs) takes FULL numpy inputs, returns FULL [N, H] f32 output.

Strategy v3 (per-layer staggered chains, bf16):
- Sort sequences by length DESC, deal round-robin across 8 cores (data parallel).
- Transposed layout: 21 groups of 6 hidden rows on partitions (126 rows + ones
  row), sequences along the free dim, rank r -> (g = r % 21, col = r // 21) so
  active columns form a shrinking prefix; per-step active width baked from the
  actual lengths at trace time.
- Layer 1 lags layer 0 by one step. Each layer runs its own dependency chain
  (own PSUM tiles, own h tile) so the two chains pipeline through the engines;
  l0's recurrence is not serialized behind l1's ops. 129 fused steps.
- All matmuls bf16 (1 cycle/row), biases folded into a ones-row at partition
  126 of the rhs. x packed RAGGED in bf16 (per-step width, offsets baked).
- Freeze past sequence end: baked mask (only the <=2 boundary columns/step)
  max'ed into the z-gate PSUM pre-sigmoid: +40 -> z=1 and u=1-z=0 exactly, so
  h' = u*n + z*h == h. Columns beyond the active width are never touched.
- GRU algebra: h' = u*n + z*h with u = 1-z and z*h computed off the critical
  path; critical chain is MM -> sig(r) -> t1 -> t2 -> tanh -> un -> h'.
"""

import math

import numpy as np
import ml_dtypes

import concourse.bacc as bacc
import concourse.bass as bass
import concourse.mybir as mybir
import concourse.tile as tile
from concourse.bass_utils import run_bass_kernel_spmd

NC = 8          # cores
G = 21          # groups per core
HP = 6          # hidden size
P = G * HP      # 126 data partitions
KP = P + 1      # +1 ones row

F32 = mybir.dt.float32
BF16 = mybir.dt.bfloat16
AF = mybir.ActivationFunctionType
ALU = mybir.AluOpType
NPBF = ml_dtypes.bfloat16


def _plan(lengths):
    """Sort desc, deal round-robin. Returns per-core rank->orig index and W schedule."""
    order = np.argsort(-lengths, kind="stable")
    percore = [order[c::NC] for c in range(NC)]          # each desc-sorted
    s = max(len(pc) for pc in percore)
    w0 = math.ceil(s / G)
    t_max = int(lengths.max())
    cnts = np.zeros((NC, t_max), np.int64)
    for c in range(NC):
        ls = lengths[percore[c]]
        for t in range(t_max):
            cnts[c, t] = int(np.count_nonzero(ls > t))
    wts = [max(1, int(np.max(np.ceil(cnts[:, t] / G)))) for t in range(t_max)]
    return percore, w0, wts, cnts


def _build_lhst(W_ih, W_hh, b_ih, b_hh, l):
    """12 block-diag lhsT matrices -> dict[(side, gate)] of [KP, P] f32."""
    out = {}
    for side in ("x", "h"):
        Wm = W_ih[l] if side == "x" else W_hh[l]          # [18, 6]
        for qi, q in enumerate(("r", "z", "n")):
            m = np.zeros((KP, P), np.float32)
            blk = Wm[qi * HP:(qi + 1) * HP, :]           # [6(out j), 6(in k)]
            for g in range(G):
                m[g * HP:(g + 1) * HP, g * HP:(g + 1) * HP] = blk.T  # [k, j]
            if side == "x":
                bias = b_ih[l][qi * HP:(qi + 1) * HP].copy()
                if q != "n":
                    bias = bias + b_hh[l][qi * HP:(qi + 1) * HP]
            else:
                bias = (b_hh[l][qi * HP:(qi + 1) * HP]
                        if q == "n" else np.zeros(HP, np.float32))
            for g in range(G):
                m[P, g * HP:(g + 1) * HP] = bias
            out[(side, q)] = m
    return out


def _fused_schedule(wts):
    """Fused step widths vf[0..T], x offsets, total x cols."""
    t_steps = len(wts)
    vf = [wts[0]] + [wts[f - 1] for f in range(1, t_steps + 1)]
    xoff = np.zeros(t_steps, np.int64)
    acc = 0
    for f in range(t_steps):
        xoff[f] = acc
        acc += vf[f]
    return vf, xoff, acc


def _mask_schedule(wts, cnts):
    """Per l1-step tt: mask region [q_tt, wts[tt]) and offsets into M."""
    t_steps = len(wts)
    qs, ms, moff = [], [], []
    acc = 0
    for tt in range(t_steps):
        q = int(min(cnts[c, tt] // G for c in range(NC)))
        m = wts[tt] - q
        qs.append(q)
        ms.append(m)
        moff.append(acc)
        acc += m
    return qs, ms, moff, acc


def _build_program(t_steps, w0, wts, vf, xoff, xcols, qs, ms, moff, mcols,
                   n_dma_slices):
    nc = bacc.Bacc(None, target_bir_lowering=False)
    x_dram = nc.declare_dram_parameter("x_sb", [KP, xcols], BF16, isOutput=False)
    lw_dram = nc.declare_dram_parameter("w_all", [KP, 13 * P], BF16, isOutput=False)
    m_dram = nc.declare_dram_parameter("mask_t", [KP, max(mcols, 1)], BF16,
                                       isOutput=False)
    ones_dram = nc.declare_dram_parameter("ones_t", [1, w0], BF16,
                                          isOutput=False)
    out_dram = nc.declare_dram_parameter("out", [P, w0], BF16, isOutput=True)

    with tile.TileContext(nc) as tc:
        with (
            tc.tile_pool(name="persist", bufs=1) as pp,
            tc.tile_pool(name="work", bufs=2) as wp,
            tc.tile_pool(name="psum", bufs=1, space=bass.MemorySpace.PSUM) as psp,
        ):
            x_sb = pp.tile([KP, xcols], BF16)
            m_sb = pp.tile([KP, max(mcols, 1)], BF16)
            h = [pp.tile([KP, w0], BF16, tag=f"h{i}", name=f"h{i}")
                 for i in range(2)]
            lw_all = pp.tile([KP, 13 * P], BF16)
            nc.sync.dma_start(lw_all[:], lw_dram[:])
            nc.sync.dma_start(m_sb[:], m_dram[:])
            lw = {}
            for l in range(2):
                for j, side in enumerate(("x", "h")):
                    for k, q in enumerate(("r", "z", "n")):
                        idx = l * 6 + j * 3 + k
                        lw[(l, side, q)] = lw_all[:, idx * P:(idx + 1) * P]
            lw_mask = lw_all[:, 12 * P:13 * P]
            sl = math.ceil(xcols / n_dma_slices)
            for i in range(n_dma_slices):
                a, b = i * sl, min(xcols, (i + 1) * sl)
                if a < b:
                    nc.sync.dma_start(x_sb[:, a:b], x_dram[:, a:b])
            for l in range(2):
                nc.gpsimd.memset(h[l][0:P, :], 0.0)
                # ones row at partition 126: engine ops need quarter-aligned
                # partition starts, so fill via DMA
                nc.sync.dma_start(h[l][P:KP, :], ones_dram[:])

            # persistent per-layer PSUM tiles; hn+xn share a bank (regions 0/1)
            ps = {(l, g): psp.tile([P, w0], F32, tag=f"ps_{g}{l}",
                                   name=f"ps_{g}{l}")
                  for l in range(2) for g in ("r", "z")}
            ps_nx = {l: psp.tile([P, 2, w0], F32, tag=f"ps_nx{l}",
                                 name=f"ps_nx{l}") for l in range(2)}
            # scratch bank for p-state filler matmuls (results unused)
            ps_fill = psp.tile([P, 48], F32, tag="ps_fill", name="ps_fill")

            for f in range(t_steps + 1):
                w = vf[f]
                layers = ([0] if f == 0 else
                          [1] if f == t_steps else [0, 1])
                # matmuls: r + hn first (they gate the chain), then xn, z
                for l in layers:
                    rhs_x = (x_sb[:, xoff[f]:xoff[f] + w] if l == 0
                             else h[0][0:KP, 0:w])
                    rhs_h = h[l][0:KP, 0:w]
                    nc.tensor.matmul(ps[(l, "r")][0:P, 0:w], lw[(l, "x", "r")][:],
                                     rhs_x, start=True, stop=False)
                    nc.tensor.matmul(ps[(l, "r")][0:P, 0:w], lw[(l, "h", "r")][:],
                                     rhs_h, start=False, stop=True)
                    nc.tensor.matmul(ps_nx[l][0:P, 0, 0:w], lw[(l, "h", "n")][:],
                                     rhs_h, start=True, stop=True)
                for l in layers:
                    rhs_x = (x_sb[:, xoff[f]:xoff[f] + w] if l == 0
                             else h[0][0:KP, 0:w])
                    rhs_h = h[l][0:KP, 0:w]
                    nc.tensor.matmul(ps_nx[l][0:P, 1, 0:w], lw[(l, "x", "n")][:],
                                     rhs_x, start=True, stop=True)
                    # freeze handled on the PE: a 13th block 40*I matmul adds
                    # +40 to frozen cells of l1's z PSUM pre-sigmoid -> z=1,
                    # u=0 exactly; keeps the mask off the Vector queue
                    zmask = l == 1 and ms[f - 1] > 0
                    nc.tensor.matmul(ps[(l, "z")][0:P, 0:w], lw[(l, "x", "z")][:],
                                     rhs_x, start=True, stop=False)
                    nc.tensor.matmul(ps[(l, "z")][0:P, 0:w], lw[(l, "h", "z")][:],
                                     rhs_h, start=False, stop=not zmask)
                    if zmask:
                        tt = f - 1
                        q, m, mo = qs[tt], ms[tt], moff[tt]
                        nc.tensor.matmul(ps[(1, "z")][0:P, q:q + m],
                                         lw_mask[:], m_sb[0:KP, mo:mo + m],
                                         start=False, stop=True)

                for l in layers:
                    r_sb = wp.tile([P, w0], BF16, tag=f"r{l}")
                    z_sb = wp.tile([P, w0], BF16, tag=f"z{l}")
                    n_sb = wp.tile([P, w0], BF16, tag=f"n{l}")
                    u_sb = wp.tile([P, w0], BF16, tag=f"u{l}")
                    t1 = wp.tile([P, w0], F32, tag=f"t1{l}")
                    t2 = wp.tile([P, w0], F32, tag=f"t2{l}")
                    zh = wp.tile([P, w0], BF16, tag=f"zh{l}")
                    un = wp.tile([P, w0], BF16, tag=f"un{l}")

                    nc.scalar.activation(r_sb[0:P, 0:w], ps[(l, "r")][0:P, 0:w],
                                         AF.Sigmoid)
                    # critical chain: t1 = r*hn, t2 = t1+xn (in-place in PSUM
                    # so tanh reads PSUM), n = tanh(t2)
                    nc.vector.tensor_mul(t1[0:P, 0:w], r_sb[0:P, 0:w],
                                         ps_nx[l][0:P, 0, 0:w])
                    nc.vector.tensor_add(ps_nx[l][0:P, 1, 0:w], t1[0:P, 0:w],
                                         ps_nx[l][0:P, 1, 0:w])
                    nc.scalar.activation(n_sb[0:P, 0:w],
                                         ps_nx[l][0:P, 1, 0:w], AF.Tanh)
                    nc.scalar.activation(z_sb[0:P, 0:w], ps[(l, "z")][0:P, 0:w],
                                         AF.Sigmoid)
                    # off-path: u = 1-z, zh = z*h (old h)
                    nc.gpsimd.tensor_scalar(u_sb[0:P, 0:w], z_sb[0:P, 0:w],
                                            -1.0, 1.0, ALU.mult, ALU.add)
                    nc.gpsimd.tensor_mul(zh[0:P, 0:w], z_sb[0:P, 0:w],
                                         h[l][0:P, 0:w])
                    # h' = u*n + z*h  (un, h' back-to-back on Vector)
                    nc.vector.tensor_mul(un[0:P, 0:w], u_sb[0:P, 0:w],
                                         n_sb[0:P, 0:w])
                    nc.vector.tensor_add(h[l][0:P, 0:w], un[0:P, 0:w],
                                         zh[0:P, 0:w])
                    if l == 0:
                        # p-state fillers: dummy matmuls gated on successive
                        # chain tiles keep the PE continuously busy so it runs
                        # at full clock; results are never read
                        fw = min(48, w)
                        for src in (r_sb, n_sb, un):
                            nc.tensor.matmul(ps_fill[0:P, 0:fw],
                                             lw_mask[0:P, 0:P],
                                             src[0:P, 0:fw],
                                             start=True, stop=True)

            nc.sync.dma_start(out_dram[:], h[1][0:P, :])
    nc.compile()
    return nc


def kernel(x, lengths, W_ih, W_hh, b_ih, b_hh):
    out, _ = kernel_traced(x=x, lengths=lengths, W_ih=W_ih, W_hh=W_hh,
                           b_ih=b_ih, b_hh=b_hh, trace=False)
    return out


def kernel_traced(x, lengths, W_ih, W_hh, b_ih, b_hh, trace=False):
    x = np.ascontiguousarray(x, np.float32)
    lengths = np.ascontiguousarray(lengths)
    n, t_dim, i_dim = x.shape
    assert i_dim == HP
    percore, w0, wts, cnts = _plan(lengths)
    t_steps = len(wts)
    vf, xoff, xcols = _fused_schedule(wts)
    qs, ms, moff, mcols = _mask_schedule(wts, cnts)

    lhst = {}
    for l in range(2):
        for k, v in _build_lhst(np.asarray(W_ih, np.float32),
                                np.asarray(W_hh, np.float32),
                                np.asarray(b_ih, np.float32),
                                np.asarray(b_hh, np.float32), l).items():
            lhst[(l,) + k] = v
    w_all = np.zeros((KP, 13 * P), np.float32)
    for l in range(2):
        for j, side in enumerate(("x", "h")):
            for k, q in enumerate(("r", "z", "n")):
                idx = l * 6 + j * 3 + k
                w_all[:, idx * P:(idx + 1) * P] = lhst[(l, side, q)]
    w_all[0:P, 12 * P:13 * P] = 40.0 * np.eye(P, dtype=np.float32)
    w_all = w_all.astype(NPBF)

    in_maps = []
    for c in range(NC):
        idx = percore[c]
        s = len(idx)
        pad = G * w0 - s
        xs = x[idx][:, :t_steps, :]                      # [s, t_steps, 6]
        if pad:
            xs = np.concatenate([xs, np.zeros((pad, t_steps, HP), np.float32)], 0)
        xg = np.empty((KP, xcols), np.float32)
        xg[P, :] = 1.0
        for f in range(t_steps):
            v = vf[f]
            blk = xs[:G * v, f, :].reshape(v, G, HP)     # [col, g, k]
            xg[0:P, xoff[f]:xoff[f] + v] = blk.transpose(1, 2, 0).reshape(P, v)
        # mask rhs: 1.0 -> frozen cell (40*I matmul adds +40 pre-sigmoid so
        # z=1, u=0 exactly), 0.0 -> active (adds nothing)
        mg = np.zeros((KP, max(mcols, 1)), np.float32)
        for tt in range(t_steps):
            q, m, mo = qs[tt], ms[tt], moff[tt]
            if m <= 0:
                continue
            cnt = int(cnts[c, tt])
            qc, kc = cnt // G, cnt % G
            for j in range(q, wts[tt]):
                col = mg[:, mo + (j - q)]
                if j > qc or (j == qc and kc == 0):
                    col[0:P] = 1.0
                elif j == qc:
                    col[kc * HP:P] = 1.0
        in_maps.append({"x_sb": xg.astype(NPBF), "w_all": w_all,
                        "mask_t": mg.astype(NPBF),
                        "ones_t": np.ones((1, w0), NPBF)})

    nc = _build_program(t_steps, w0, wts, vf, xoff, xcols, qs, ms, moff, mcols,
                        n_dma_slices=10)
    bkr = run_bass_kernel_spmd(nc, in_maps, list(range(NC)), trace=trace)
    res = bkr.results

    out = np.zeros((n, HP), np.float32)
    for c in range(NC):
        idx = percore[c]
        og = np.asarray(res[c]["out"], dtype=np.float32)  # [P, w0]
        vals = og.reshape(G, HP, w0).transpose(2, 0, 1).reshape(G * w0, HP)
        out[idx] = vals[:len(idx)]
    return out, bkr
